# revision 68
# baseline (speedup 1.0000x reference)
"""HCR layer (tensor-product Legendre basis -> dense projection) on 8 trn2 cores.

Math: density[b,o] = 1 + sum_f Bfull[b,f] * C[o,f] - C[o,0]
  where Bfull[b, (i,j,k)] = Li(x0)*Lj(x1)*Lk(x2), orthonormal Legendre on [0,1],
  degree 15 -> 16^3 = 4096 features, batch 8192, out 1024.

Since f_0 == 1 exactly, Bfull[:,0] == 1, so with C[:,0] replaced by 1.0 the
plain matmul Bfull @ C'^T equals the final density (the +1 and the -C[o,0]
fold into the feature-0 column).

Precision plan: the bulk matmul runs in fp8 e4m3 with DoubleRow perf mode
(2 fp8 K-values per partition-cycle -> 2x the fp16 rate, 157 TF/s/core,
measured 214ns per K=256 x 512-col matmul). Plain e4m3 on both operands gives
max rel err 3.4e-2 (tolerance 2e-2); the excess error is concentrated in rows
whose Legendre basis has large norm (x near the domain corners). Per batch
shard, the KSEL=128 rows with the largest row energy prod_d sum_k L[b,d,k]^2
(an exact formula for ||Bfull_row||^2) are recomputed in fp16 in a small
second pass and overwritten at host assembly. Measured residual: max rel err
1.619e-2, bit-identical to the ml_dtypes e4m3 simulation.

Sharding: batch 4-way x out 2-way = 8 cores, no communication.
Per core: [2048 batch, 512 out, 4096 feat] in fp8 (16 DoubleRow matmuls of
K=256 per PSUM bank, 2 batch-half passes of 8 banks, 54.8us at peak) +
[128 sel rows, 512 out, 4096 feat] in fp16 (sel rows stationary, C moving:
2 column-half chains of 32 x 256-col matmuls, 6.8us).

Schedule: one in-order sync-HWDGE queue streams all inputs (a single queue
aggregates the 16 HW DMA engines at ~358 GB/s; each dma_start costs ~650ns
of issue time, so transfers are coarse: one combined ct+bf transfer per
K-pair, pair 0 split in two halves). While the first transfer lands, the PE
runs full-width zeroing junk matmuls that double as accumulation-group
pre-opens for the 8 pass-0 banks (start=False on the data-gated matmuls) and
ramp the PE clock. Pass 0 runs pair-outer (consumes each pair as it lands);
pass 1 runs bank-outer so accumulator stops stagger and each bank's
PSUM->SBUF fp16 copy (scalar) + output DMA (gpsimd SW-DGE, with a 1-elem
gpsimd read absorbing the copy wait) hide behind the matmul stream.
"""

from contextlib import ExitStack

import ml_dtypes
import numpy as np

import concourse.bass as bass
import concourse.mybir as mybir
import concourse.tile as tile
from concourse.bass_utils import run_bass_kernel_spmd

M = 15
NDEG = M + 1            # 16
OUT = 1024
BATCH = 8192
NFEAT = NDEG ** 3       # 4096
NB = 4                  # batch shards
NO = 2                  # out shards
BC = BATCH // NB        # 2048 batch per core
OC = OUT // NO          # 512 out per core
KT = NFEAT // 128       # 32 contraction tiles of 128
NPAIR = KT // 2         # 16 DoubleRow pairs of K=256
BH = BC // 2            # 1024: batch half processed per fp8 pass
KSEL = 128              # corrected rows per batch shard (fp16 pass)
FP8 = mybir.dt.float8e4
FP16 = mybir.dt.float16
FP32 = mybir.dt.float32
DR = mybir.MatmulPerfMode.DoubleRow

_cache = {}


class _SplitDrainTileContext(tile.TileContext):
    """TRN2 allows few sem waits per instruction; the default kernel-tail
    drain carries one wait per ticked proc and fails walrus codegen. Split
    the waits across a chain of drains on the sync engine."""

    _MAXW = 1

    def _drain_and_barrier(self, tick_clock, wait_clock):
        from concourse.vector_clock import ScopedClock

        nc = self.nc
        drain0 = nc.sync.drain()
        wait_clock.add_sem_waits(
            drain0.ins, ScopedClock({None: tick_clock.global_clock})
        )
        si = drain0.ins.sync_info
        waits = list(si.on_wait) if si and si.on_wait else []
        if len(waits) > self._MAXW:
            drain0.ins.sync_info = mybir.SyncInfo(
                on_wait=waits[: self._MAXW],
                on_update=list(si.on_update) if si.on_update else [],
            )
            for i in range(self._MAXW, len(waits), self._MAXW):
                d = nc.sync.drain()
                d.ins.sync_info = mybir.SyncInfo(
                    on_wait=waits[i : i + self._MAXW], on_update=[]
                )

        nc.all_engine_barrier()
        assert self.sems is not None
        popped = nc._tile_sem_poison_stack.pop()
        assert popped is self._sem_poison
        # Skip clear_and_free_semaphores + the second barrier: the walrus
        # codegen epilogue zeroes the whole 256-sem file (ids 7-255) on
        # every engine anyway, so the bass-level range-clear and its
        # closing barrier only add ~1.5us to the measured tail.


def _legendre_basis_np(x):
    """Match reference fp32 recurrence exactly. x: [B, D] fp32 -> [B, D, 16]."""
    t = 2.0 * x - 1.0
    ps = [np.ones_like(t), t]
    for k in range(1, M):
        ps.append(((2 * k + 1) * t * ps[k] - k * ps[k - 1]) / (k + 1))
    ps = ps[: M + 1]
    scale = np.sqrt(2.0 * np.arange(M + 1, dtype=x.dtype) + 1.0)
    return np.stack(ps, axis=-1) * scale


def _build_program():
    if "nc" in _cache:
        return _cache["nc"]

    nc = bass.Bass(
        "TRN2", target_bir_lowering=False, debug=False, num_devices=NB * NO
    )

    # Partition-major packed inputs (see _make_in_maps for layouts):
    # q8:  pair-0 combined halves [128, 2, 4, 512]: half h = 2 ct rows
    #      (256 data + 256 pad each) + 2 bf rows; h = (ot-half, b2) pairing
    # cb8: pairs 1..15 combined [128, 15, 6, 512]: 2 ct rows + 4 bf rows
    #      (b2-major) per pair -- one DMA delivers a whole pair
    # bf8: batch-half-1 basis [128, KT, BH] (kt-major)
    # ct16/bs16: kt tiles [128, OC] / [128, KSEL] fp16 at cols kt*OC / kt*KSEL
    q8_d = nc.dram_tensor("q8", [128, 8, 512], FP8, kind="ExternalInput").ap()
    cb8_d = nc.dram_tensor(
        "cb8", [128, (NPAIR - 1) * 6, 512], FP8, kind="ExternalInput"
    ).ap()
    bf8_d = nc.dram_tensor("bf8", [128, KT, BH], FP8, kind="ExternalInput").ap()
    ct16_d = nc.dram_tensor("ct16", [128, KT * OC], FP16, kind="ExternalInput").ap()
    bs16_d = nc.dram_tensor("bs16", [128, KT * KSEL], FP16, kind="ExternalInput").ap()
    # 16 bank dumps [128 out, 512 batch] fp16, g = pass*8 + ot*2 + b2
    out_d = nc.dram_tensor("out16", [16 * 128, 512], FP16, kind="ExternalOutput").ap()
    # correction dump [KSEL rows, 512 out] fp16
    outc_d = nc.dram_tensor("outc16", [KSEL, OC], FP16, kind="ExternalOutput").ap()

    with _SplitDrainTileContext(nc) as tc, ExitStack() as ctx:
        jkp = ctx.enter_context(tc.tile_pool(name="jkp", bufs=1))
        ctp8 = ctx.enter_context(tc.tile_pool(name="ctp8", bufs=NPAIR // 2))
        bfp8 = ctx.enter_context(tc.tile_pool(name="bfp8", bufs=NPAIR // 2 + 1))
        ctp16 = ctx.enter_context(tc.tile_pool(name="ctp16", bufs=1))
        bsp16 = ctx.enter_context(tc.tile_pool(name="bsp16", bufs=1))
        psp = ctx.enter_context(tc.tile_pool(name="psp", bufs=8, space="PSUM"))
        outp = ctx.enter_context(tc.tile_pool(name="outp", bufs=21))

        scratch = outp.tile([1, 32], FP16, tag="scratch", name="scratch", bufs=1)

        # --- PE warmup doubles as PSUM pre-open: full-width zeroing junk
        # matmuls (junk is memset 0) open all 8 pass-0 accumulation banks
        # while the first tiles stream in, so no data-gated matmul pays the
        # accumulation-start cost and the PE clock ramps on real-sized work.
        junk = jkp.tile([128, 512], FP16, tag="junk", name="junk")
        nc.vector.memset(junk[:], 0)

        # --- Input DMAs, all on the sync HWDGE queue (one in-order stream).
        # Each dma_start costs ~650ns of issue time on the queue engine, so
        # DMAs are coarse: 2-pair chunks (ct8 interleaved with bf8 half 0)
        # feed the streaming pass 0 with ~2.2us first-data latency, then the
        # rest of the inputs ride in 4 big transfers. The in-order queue
        # naturally prioritizes the latency-critical stream front. ---
        q8_sb = [
            ctp8.tile([128, 4, 512], FP8, tag="q8", name=f"q8_{h}", bufs=2)
            for h in range(2)
        ]
        cb8_sb = [
            bfp8.tile(
                [128, 6, 512], FP8, tag="cb8", name=f"cb8_{p}", bufs=NPAIR - 1
            )
            for p in range(NPAIR - 1)
        ]
        bf81_sb = [
            bfp8.tile([128, 8, BH], FP8, tag="bf81", name=f"bf81_{c}", bufs=4)
            for c in range(4)
        ]
        ct16_sb = ctp16.tile([128, KT * OC], FP16, tag="ct16", name="ct16", bufs=1)
        bs16_sb = bsp16.tile([128, KT * KSEL], FP16, tag="bs16", name="bs16", bufs=1)

        # A single in-order sync queue aggregates all 16 HW DMA engines at
        # ~358 GB/s; splitting across two HWDGE queues halves per-transfer
        # speed (measured), so everything streams here, latency-critical
        # tiles first. One DMA delivers a whole pair (ct+bf combined), and
        # pair 0 rides in two half-transfers so the first matmul is gated
        # on a single 256KB transfer.
        nc.sync.dma_start(out=q8_sb[0][:], in_=q8_d[:, 0:4, :])
        nc.sync.dma_start(out=q8_sb[1][:], in_=q8_d[:, 4:8, :])
        for p in range(NPAIR - 1):
            nc.sync.dma_start(
                out=cb8_sb[p][:], in_=cb8_d[:, p * 6 : (p + 1) * 6, :]
            )
        for c in range(4):
            nc.sync.dma_start(
                out=bf81_sb[c][:], in_=bf8_d[:, c * 8 : (c + 1) * 8, :]
            )
        nc.sync.dma_start(out=ct16_sb[:], in_=ct16_d[:])
        nc.sync.dma_start(out=bs16_sb[:], in_=bs16_d[:])

        def ct8_ap(g, ot):
            if g == 0:
                return q8_sb[ot // 2][:, 0:2, (ot % 2) * 128 : (ot % 2 + 1) * 128]
            return cb8_sb[g - 1][:, 0:2, ot * 128 : (ot + 1) * 128]

        def bf8_ap(h, g, b2):
            if h == 0:
                if g == 0:
                    return q8_sb[b2][:, 2:4, :]
                return cb8_sb[g - 1][:, 2 + 2 * b2 : 4 + 2 * b2, :]
            c, j = divmod(g, 4)
            return bf81_sb[c][:, j * 2 : j * 2 + 2, b2 * 512 : (b2 + 1) * 512]

        # --- Pass 0 (fp8 DoubleRow): pair-outer, consume bf8[0] as it lands.
        ps0 = [
            psp.tile([128, 512], FP32, tag="ps", name=f"ps0_{g}") for g in range(8)
        ]
        # banks ordered by first real use (order0 below) so the pre-open
        # chain only ever delays the bank about to be consumed
        preopened = (0, 2, 4, 6, 1, 3, 5, 7)
        for bank in preopened:
            nc.tensor.matmul(
                ps0[bank][:],
                lhsT=junk[:, 0:128],
                rhs=junk[:, 0:512],
                start=True,
                stop=False,
            )
        # pair 0 follows the quarter arrival order (ctA,bfA,ctB,bfB); later
        # pairs run ot-outer over fully-landed chunks.
        order0 = [(0, 0), (1, 0), (2, 0), (3, 0), (0, 1), (1, 1), (2, 1), (3, 1)]
        for g in range(NPAIR):
            order = order0 if g == 0 else [
                (ot, b2) for ot in range(4) for b2 in range(2)
            ]
            for ot, b2 in order:
                bank = ot * 2 + b2
                nc.tensor.matmul(
                    ps0[bank][:],
                    lhsT=ct8_ap(g, ot),
                    rhs=bf8_ap(0, g, b2),
                    start=(g == 0 and bank not in preopened),
                    stop=(g == NPAIR - 1),
                    perf_mode=DR,
                )

        # Drain pass 0: ACT copies PSUM fp32 -> SBUF fp16; a 1-elem gpsimd
        # read absorbs the ACT wait onto the gpsimd stream so the SW-DGE
        # output DMA carries only its queue sem (HWDGE DMAs have one wait
        # slot and the ring wait occupies it).
        o0 = [
            outp.tile([128, 512], FP16, tag="osb", name=f"o0_{g}") for g in range(8)
        ]
        for g in range(8):
            nc.scalar.copy(o0[g][:], ps0[g][:])
            nc.gpsimd.tensor_copy(scratch[:, g : g + 1], o0[g][0:1, 0:1])
            nc.gpsimd.dma_start(
                out=out_d[g * 128 : (g + 1) * 128, :], in_=o0[g][:]
            )

        # --- Pass 1 (fp8 DoubleRow): bank-outer so stops stagger and drains
        # overlap the matmul stream. bf8[1] is resident well before needed.
        nc.tensor.ldweights(bf81_sb[0][:, 0:1, 0:128])  # absorb bf8_1 DMA wait
        ps1 = [
            psp.tile([128, 512], FP32, tag="ps", name=f"ps1_{g}") for g in range(8)
        ]
        o1 = [
            outp.tile([128, 512], FP16, tag="osb", name=f"o1_{g}") for g in range(8)
        ]
        for g8 in range(8):
            ot, b2 = divmod(g8, 2)
            for g in range(NPAIR):
                nc.tensor.matmul(
                    ps1[g8][:],
                    lhsT=ct8_ap(g, ot),
                    rhs=bf8_ap(1, g, b2),
                    start=(g == 0),
                    stop=(g == NPAIR - 1),
                    perf_mode=DR,
                )
            nc.scalar.copy(o1[g8][:], ps1[g8][:])
            nc.gpsimd.tensor_copy(scratch[:, 8 + g8 : 9 + g8], o1[g8][0:1, 0:1])
            nc.gpsimd.dma_start(
                out=out_d[(8 + g8) * 128 : (9 + g8) * 128, :], in_=o1[g8][:]
            )

        # --- Correction pass (fp16): the KSEL=128 selected rows are the
        # stationary operand and C the moving one; the output lands as
        # [sel row, out] directly. Two sequential column-half chains so the
        # first half's drain overlaps the second half's matmuls.
        nc.tensor.ldweights(ct16_sb[:, 0:128])  # absorb ct16 DMA wait
        nc.tensor.ldweights(bs16_sb[:, 0:128])  # absorb bs16 DMA wait
        psc = [
            psp.tile([128, 512], FP32, tag="ps", name=f"psc_{hh}") for hh in range(2)
        ]
        oc = outp.tile([128, OC], FP16, tag="osb", name="oc")
        for hh in range(2):
            cs = slice(hh * (OC // 2), (hh + 1) * (OC // 2))
            for k in range(KT):
                nc.tensor.matmul(
                    psc[hh][:, 0 : OC // 2],
                    lhsT=bs16_sb[:, k * KSEL : (k + 1) * KSEL],
                    rhs=ct16_sb[:, k * OC + hh * (OC // 2) : k * OC + (hh + 1) * (OC // 2)],
                    start=(k == 0),
                    stop=(k == KT - 1),
                )
            nc.scalar.copy(oc[:, cs], psc[hh][:, 0 : OC // 2])
            nc.gpsimd.tensor_copy(scratch[:, 16 + hh : 17 + hh], oc[0:1, cs][:, 0:1])
            nc.gpsimd.dma_start(out=outc_d[:, cs], in_=oc[:, cs])

    _cache["nc"] = nc
    return nc


def _make_in_maps(x, coefficients):
    L = _legendre_basis_np(np.asarray(x, dtype=np.float32))  # [8192, 3, 16]
    # exact ||Bfull_row||^2 = prod_d sum_k L[b,d,k]^2: ranks rows by fp8
    # quantization-error magnitude
    pred = (L.astype(np.float64) ** 2).sum(axis=2).prod(axis=1)
    sel = [
        np.sort(np.argsort(-pred[bs * BC : (bs + 1) * BC])[:KSEL])
        for bs in range(NB)
    ]

    CT = np.ascontiguousarray(np.asarray(coefficients, dtype=np.float32).T)
    CT[0, :] = 1.0  # folds both the +1 and the -C[:,0] term (Bfull[:,0]==1)
    CT8 = CT.astype(ml_dtypes.float8_e4m3)  # TRN e4m3 (max 240); |C| < 6
    CT16 = CT.astype(np.float16)

    in_maps = []
    for c in range(NB * NO):
        bs, osh = c % NB, c // NB
        Lb = L[bs * BC : (bs + 1) * BC]  # [BC, 3, 16]
        bfull = np.einsum("bi,bj,bk->ijkb", Lb[:, 0], Lb[:, 1], Lb[:, 2])
        bfull = bfull.reshape(NFEAT, BC)
        bf8 = bfull.astype(ml_dtypes.float8_e4m3)
        # kt-major partition packs: [KT or 2KT, 128, cols] -> [128, kt, cols]
        ctp = (
            CT8[:, osh * OC : (osh + 1) * OC]
            .reshape(KT, 128, OC)
            .transpose(1, 0, 2)
        )  # [128, 32, 512]
        bfp = bf8.reshape(KT, 128, BC).transpose(1, 0, 2)  # [128, 32, 2048]

        # q8: pair-0 combined halves [128, 2(half), 4(row), 512]
        # half h: rows 0-1 = ct s0,s1 cols h*256:(h+1)*256 (padded to 512),
        #         rows 2-3 = bf s0,s1 batch cols h*512:(h+1)*512 of half 0
        q8 = np.zeros((128, 2, 4, 512), dtype=ml_dtypes.float8_e4m3)
        for h in range(2):
            q8[:, h, 0:2, 0:256] = ctp[:, 0:2, h * 256 : (h + 1) * 256]
            q8[:, h, 2:4, :] = bfp[:, 0:2, h * 512 : (h + 1) * 512]
        q8 = q8.reshape(128, 8, 512)

        # cb8: pairs 1..15 [128, pair, 6, 512]: rows 0-1 ct s0,s1; rows 2-5
        # bf (b2-major, s inner) of batch half 0
        cb8 = np.empty((128, NPAIR - 1, 6, 512), dtype=ml_dtypes.float8_e4m3)
        for p in range(1, NPAIR):
            cb8[:, p - 1, 0:2, :] = ctp[:, 2 * p : 2 * p + 2, :]
            blk = bfp[:, 2 * p : 2 * p + 2, 0:BH]  # [128, 2(s), 1024]
            cb8[:, p - 1, 2:6, :] = (
                blk.reshape(128, 2, 2, 512).transpose(0, 2, 1, 3).reshape(128, 4, 512)
            )
        cb8 = cb8.reshape(128, (NPAIR - 1) * 6, 512)

        # bf8: batch half 1, kt-major [128, 32, 1024]
        bpk1 = np.ascontiguousarray(bfp[:, :, BH:])

        bsel = np.ascontiguousarray(bfull[:, sel[bs]]).astype(np.float16)
        bspk = np.ascontiguousarray(
            bsel.reshape(KT, 128, KSEL).transpose(1, 0, 2).reshape(128, -1)
        )
        slab16 = CT16[:, osh * OC : (osh + 1) * OC]
        cpk16 = np.ascontiguousarray(
            slab16.reshape(KT, 128, OC).transpose(1, 0, 2).reshape(128, -1)
        )
        in_maps.append(
            {
                "q8": np.ascontiguousarray(q8),
                "cb8": np.ascontiguousarray(cb8),
                "bf8": bpk1,
                "ct16": cpk16,
                "bs16": bspk,
            }
        )
    return in_maps, sel


def _assemble(results, sel):
    out = np.empty((BATCH, OUT), dtype=np.float32)
    for c in range(NB * NO):
        bs, osh = c % NB, c // NB
        blk = results[c]["out16"].reshape(2, 4, 2, 128, 512)  # [pass, ot, b2, o, b]
        core = np.ascontiguousarray(
            blk.transpose(0, 2, 4, 1, 3).reshape(BC, OC)
        ).astype(np.float32)
        out[bs * BC : (bs + 1) * BC, osh * OC : (osh + 1) * OC] = core
    for c in range(NB * NO):
        bs, osh = c % NB, c // NB
        corr = results[c]["outc16"]  # [sel row, out]
        out[bs * BC + sel[bs], osh * OC : (osh + 1) * OC] = corr.astype(np.float32)
    return out


def _run(x, coefficients, trace=False, **kwargs):
    nc = _build_program()
    in_maps, sel = _make_in_maps(x, coefficients)
    res = run_bass_kernel_spmd(
        nc, in_maps, list(range(NB * NO)), trace=trace, **kwargs
    )
    return _assemble(res.results, sel), res


def kernel(x, coefficients):
    out, _ = _run(x, coefficients)
    return out


# revision 85
# speedup vs baseline: 1.0185x; 1.0185x over previous
"""HCR layer (tensor-product Legendre basis -> dense projection) on 8 trn2 cores.

Math: density[b,o] = 1 + sum_f Bfull[b,f] * C[o,f] - C[o,0]
  where Bfull[b, (i,j,k)] = Li(x0)*Lj(x1)*Lk(x2), orthonormal Legendre on [0,1],
  degree 15 -> 16^3 = 4096 features, batch 8192, out 1024.

Since f_0 == 1 exactly, Bfull[:,0] == 1, so with C[:,0] replaced by 1.0 the
plain matmul Bfull @ C'^T equals the final density (the +1 and the -C[o,0]
fold into the feature-0 column).

Precision plan: the bulk matmul runs in fp8 e4m3 with DoubleRow perf mode
(2 fp8 K-values per partition-cycle -> 2x the fp16 rate, 157 TF/s/core,
measured 214ns per K=256 x 512-col matmul). Plain e4m3 on both operands gives
max rel err 3.4e-2 (tolerance 2e-2); the excess error is concentrated in rows
whose Legendre basis has large norm (x near the domain corners). Per batch
shard, the KSEL=128 rows with the largest row energy prod_d sum_k L[b,d,k]^2
(an exact formula for ||Bfull_row||^2) are recomputed in fp16 in a small
second pass and overwritten at host assembly. Measured residual: max rel err
1.619e-2, bit-identical to the ml_dtypes e4m3 simulation.

Sharding: batch 4-way x out 2-way = 8 cores, no communication.
Per core: [2048 batch, 512 out, 4096 feat] in fp8 (16 DoubleRow matmuls of
K=256 per PSUM bank, 2 batch-half passes of 8 banks, 54.8us at peak) +
[128 sel rows, 512 out, 4096 feat] in fp16 (sel rows stationary, C moving:
2 column-half chains of 32 x 256-col matmuls, 6.8us).

Schedule: one in-order sync-HWDGE queue streams all inputs (a single queue
aggregates the 16 HW DMA engines at ~358 GB/s; each dma_start costs ~650ns
of issue time, so transfers are coarse: one combined ct+bf transfer per
K-pair, pair 0 split in two halves). While the first transfer lands, the PE
runs full-width zeroing junk matmuls that double as accumulation-group
pre-opens for the 8 pass-0 banks (start=False on the data-gated matmuls) and
ramp the PE clock. Pass 0 runs pair-outer (consumes each pair as it lands);
pass 1 runs bank-outer so accumulator stops stagger and each bank's
PSUM->SBUF fp16 copy (scalar) + output DMA (gpsimd SW-DGE, with a 1-elem
gpsimd read absorbing the copy wait) hide behind the matmul stream.
"""

from contextlib import ExitStack

import ml_dtypes
import numpy as np

import concourse.bass as bass
import concourse.mybir as mybir
import concourse.tile as tile
from concourse.bass_utils import run_bass_kernel_spmd

M = 15
NDEG = M + 1            # 16
OUT = 1024
BATCH = 8192
NFEAT = NDEG ** 3       # 4096
NB = 4                  # batch shards
NO = 2                  # out shards
BC = BATCH // NB        # 2048 batch per core
OC = OUT // NO          # 512 out per core
KT = NFEAT // 128       # 32 contraction tiles of 128
NPAIR = KT // 2         # 16 DoubleRow pairs of K=256
KSEL = 128              # corrected rows per batch shard (fp16 pass)
BF8N = BC - KSEL        # 1920: rows the fp8 pass computes (selected rows are
                        # recomputed in fp16 anyway, so fp8 skips them)
BH = BF8N // 2          # 960: batch half per fp8 pass
BW = (512, 448)         # b2-tile widths within a 960 half
FP8 = mybir.dt.float8e4
FP16 = mybir.dt.float16
FP32 = mybir.dt.float32
DR = mybir.MatmulPerfMode.DoubleRow

_cache = {}


class _SplitDrainTileContext(tile.TileContext):
    """TRN2 allows few sem waits per instruction; the default kernel-tail
    drain carries one wait per ticked proc and fails walrus codegen. Split
    the waits across a chain of drains on the sync engine."""

    _MAXW = 1

    def _drain_and_barrier(self, tick_clock, wait_clock):
        from concourse.vector_clock import ScopedClock

        nc = self.nc
        drain0 = nc.sync.drain()
        wait_clock.add_sem_waits(
            drain0.ins, ScopedClock({None: tick_clock.global_clock})
        )
        si = drain0.ins.sync_info
        waits = list(si.on_wait) if si and si.on_wait else []
        if len(waits) > self._MAXW:
            drain0.ins.sync_info = mybir.SyncInfo(
                on_wait=waits[: self._MAXW],
                on_update=list(si.on_update) if si.on_update else [],
            )
            for i in range(self._MAXW, len(waits), self._MAXW):
                d = nc.sync.drain()
                d.ins.sync_info = mybir.SyncInfo(
                    on_wait=waits[i : i + self._MAXW], on_update=[]
                )

        nc.all_engine_barrier()
        assert self.sems is not None
        popped = nc._tile_sem_poison_stack.pop()
        assert popped is self._sem_poison
        # Skip clear_and_free_semaphores + the second barrier: the walrus
        # codegen epilogue zeroes the whole 256-sem file (ids 7-255) on
        # every engine anyway, so the bass-level range-clear and its
        # closing barrier only add ~1.5us to the measured tail.


def _legendre_basis_np(x):
    """Match reference fp32 recurrence exactly. x: [B, D] fp32 -> [B, D, 16]."""
    t = 2.0 * x - 1.0
    ps = [np.ones_like(t), t]
    for k in range(1, M):
        ps.append(((2 * k + 1) * t * ps[k] - k * ps[k - 1]) / (k + 1))
    ps = ps[: M + 1]
    scale = np.sqrt(2.0 * np.arange(M + 1, dtype=x.dtype) + 1.0)
    return np.stack(ps, axis=-1) * scale


def _build_program():
    if "nc" in _cache:
        return _cache["nc"]

    nc = bass.Bass(
        "TRN2", target_bir_lowering=False, debug=False, num_devices=NB * NO
    )

    # Partition-major packed inputs (see _make_in_maps for layouts):
    # q8:  pair-0 combined halves [128, 2, 4, 512]: half h = 2 ct rows
    #      (256 data + 256 pad each) + 2 bf rows; h = (ot-half, b2) pairing
    # cb8: pairs 1..15 combined [128, 15, 6, 512]: 2 ct rows + 4 bf rows
    #      (b2-major) per pair -- one DMA delivers a whole pair
    # bf8: batch-half-1 basis [128, KT, BH] (kt-major)
    # ct16/bs16: kt tiles [128, OC] / [128, KSEL] fp16 at cols kt*OC / kt*KSEL
    q8_d = nc.dram_tensor("q8", [128, 8, 512], FP8, kind="ExternalInput").ap()
    cb8_d = nc.dram_tensor(
        "cb8", [128, (NPAIR - 1) * 6, 512], FP8, kind="ExternalInput"
    ).ap()
    # bf8 (batch half 1) is zero-padded to 1024 cols: pass-1 banks must be
    # written full-width (narrow writes over reused PSUM buffers fracture
    # the WAR dep into 2 sems, over the matmul wait limit), so 64 phantom
    # zero rows ride along and the host ignores their output columns.
    bf8_d = nc.dram_tensor("bf8", [128, KT, 1024], FP8, kind="ExternalInput").ap()
    # (q8/cb8 bf rows for the b2=1 tile hold 448 data cols + 64 pad)
    ct16_d = nc.dram_tensor("ct16", [128, KT * OC], FP16, kind="ExternalInput").ap()
    bs16_d = nc.dram_tensor("bs16", [128, KT * KSEL], FP16, kind="ExternalInput").ap()
    # 16 bank dumps [128 out, 512 batch] fp16, g = pass*8 + ot*2 + b2
    out_d = nc.dram_tensor("out16", [16 * 128, 512], FP16, kind="ExternalOutput").ap()
    # correction dump [KSEL rows, 512 out] fp16
    outc_d = nc.dram_tensor("outc16", [KSEL, OC], FP16, kind="ExternalOutput").ap()

    with _SplitDrainTileContext(nc) as tc, ExitStack() as ctx:
        jkp = ctx.enter_context(tc.tile_pool(name="jkp", bufs=1))
        ctp8 = ctx.enter_context(tc.tile_pool(name="ctp8", bufs=NPAIR // 2))
        bfp8 = ctx.enter_context(tc.tile_pool(name="bfp8", bufs=NPAIR // 2 + 1))
        ctp16 = ctx.enter_context(tc.tile_pool(name="ctp16", bufs=1))
        bsp16 = ctx.enter_context(tc.tile_pool(name="bsp16", bufs=1))
        psp = ctx.enter_context(tc.tile_pool(name="psp", bufs=8, space="PSUM"))
        outp = ctx.enter_context(tc.tile_pool(name="outp", bufs=21))

        scratch = outp.tile([1, 32], FP16, tag="scratch", name="scratch", bufs=1)

        # --- PE warmup doubles as PSUM pre-open: full-width zeroing junk
        # matmuls (junk is memset 0) open all 8 pass-0 accumulation banks
        # while the first tiles stream in, so no data-gated matmul pays the
        # accumulation-start cost and the PE clock ramps on real-sized work.
        junk = jkp.tile([128, 512], FP16, tag="junk", name="junk")
        nc.vector.memset(junk[:], 0)

        # --- Input DMAs, all on the sync HWDGE queue (one in-order stream).
        # Each dma_start costs ~650ns of issue time on the queue engine, so
        # DMAs are coarse: 2-pair chunks (ct8 interleaved with bf8 half 0)
        # feed the streaming pass 0 with ~2.2us first-data latency, then the
        # rest of the inputs ride in 4 big transfers. The in-order queue
        # naturally prioritizes the latency-critical stream front. ---
        q8_sb = [
            ctp8.tile([128, 4, 512], FP8, tag="q8", name=f"q8_{h}", bufs=2)
            for h in range(2)
        ]
        cb8_sb = [
            bfp8.tile(
                [128, 6, 512], FP8, tag="cb8", name=f"cb8_{p}", bufs=NPAIR - 1
            )
            for p in range(NPAIR - 1)
        ]
        bf81_sb = [
            bfp8.tile([128, 8, 1024], FP8, tag="bf81", name=f"bf81_{c}", bufs=4)
            for c in range(4)
        ]
        ct16_sb = ctp16.tile([128, KT * OC], FP16, tag="ct16", name="ct16", bufs=1)
        bs16_sb = bsp16.tile([128, KT * KSEL], FP16, tag="bs16", name="bs16", bufs=1)

        # A single in-order sync queue aggregates all 16 HW DMA engines at
        # ~358 GB/s; splitting across two HWDGE queues halves per-transfer
        # speed (measured), so everything streams here, latency-critical
        # tiles first. One DMA delivers a whole pair (ct+bf combined), and
        # pair 0 rides in two half-transfers so the first matmul is gated
        # on a single 256KB transfer.
        nc.sync.dma_start(out=q8_sb[0][:], in_=q8_d[:, 0:4, :])
        nc.sync.dma_start(out=q8_sb[1][:], in_=q8_d[:, 4:8, :])
        for p in range(NPAIR - 1):
            nc.sync.dma_start(
                out=cb8_sb[p][:], in_=cb8_d[:, p * 6 : (p + 1) * 6, :]
            )
        for c in range(4):
            nc.sync.dma_start(
                out=bf81_sb[c][:], in_=bf8_d[:, c * 8 : (c + 1) * 8, :]
            )
        nc.sync.dma_start(out=ct16_sb[:], in_=ct16_d[:])
        nc.sync.dma_start(out=bs16_sb[:], in_=bs16_d[:])

        def ct8_ap(g, ot):
            if g == 0:
                return q8_sb[ot // 2][:, 0:2, (ot % 2) * 128 : (ot % 2 + 1) * 128]
            return cb8_sb[g - 1][:, 0:2, ot * 128 : (ot + 1) * 128]

        def bf8_ap(h, g, b2):
            if h == 0:
                w = BW[b2]
                if g == 0:
                    return q8_sb[b2][:, 2:4, 0:w]
                return cb8_sb[g - 1][:, 2 + 2 * b2 : 4 + 2 * b2, 0:w]
            c, j = divmod(g, 4)
            return bf81_sb[c][:, j * 2 : j * 2 + 2, b2 * 512 : (b2 + 1) * 512]

        # --- Pass 0 (fp8 DoubleRow): pair-outer, consume bf8[0] as it lands.
        ps0 = [
            psp.tile([128, 512], FP32, tag="ps", name=f"ps0_{g}") for g in range(8)
        ]
        # banks ordered by first real use (order0 below) so the pre-open
        # chain only ever delays the bank about to be consumed
        preopened = (0, 2, 4, 6, 1, 3, 5, 7)
        for bank in preopened:
            nc.tensor.matmul(
                ps0[bank][:],
                lhsT=junk[:, 0:128],
                rhs=junk[:, 0:512],
                start=True,
                stop=False,
            )
        # pair 0 follows the quarter arrival order (ctA,bfA,ctB,bfB); later
        # pairs run ot-outer over fully-landed chunks.
        order0 = [(0, 0), (1, 0), (2, 0), (3, 0), (0, 1), (1, 1), (2, 1), (3, 1)]
        for g in range(NPAIR):
            order = order0 if g == 0 else [
                (ot, b2) for ot in range(4) for b2 in range(2)
            ]
            for ot, b2 in order:
                bank = ot * 2 + b2
                nc.tensor.matmul(
                    ps0[bank][:, 0 : BW[b2]],
                    lhsT=ct8_ap(g, ot),
                    rhs=bf8_ap(0, g, b2),
                    start=(g == 0 and bank not in preopened),
                    stop=(g == NPAIR - 1),
                    perf_mode=DR,
                )

        # Drain pass 0: ACT copies PSUM fp32 -> SBUF fp16; a 1-elem gpsimd
        # read absorbs the ACT wait onto the gpsimd stream so the SW-DGE
        # output DMA carries only its queue sem (HWDGE DMAs have one wait
        # slot and the ring wait occupies it).
        o0 = [
            outp.tile([128, 512], FP16, tag="osb", name=f"o0_{g}") for g in range(8)
        ]
        # copies/DMAs stay full 512-wide even for 448-col banks: the spare
        # columns carry junk the host ignores, and a full-width read keeps
        # the PSUM WAR dep a single semaphore for the pass-1 bank starts
        for g in range(8):
            nc.scalar.copy(o0[g][:], ps0[g][:])
            nc.gpsimd.tensor_copy(scratch[:, g : g + 1], o0[g][0:1, 0:1])
            nc.gpsimd.dma_start(
                out=out_d[g * 128 : (g + 1) * 128, :], in_=o0[g][:]
            )

        # --- Pass 1 (fp8 DoubleRow): bank-outer so stops stagger and drains
        # overlap the matmul stream. bf8[1] is resident well before needed.
        nc.tensor.ldweights(bf81_sb[0][:, 0:1, 0:128])  # absorb bf8_1 DMA wait
        ps1 = [
            psp.tile([128, 512], FP32, tag="ps", name=f"ps1_{g}") for g in range(8)
        ]
        o1 = [
            outp.tile([128, 512], FP16, tag="osb", name=f"o1_{g}") for g in range(8)
        ]
        for g8 in range(8):
            ot, b2 = divmod(g8, 2)
            # a throwaway weight load reading the pass-0 drain output absorbs
            # the copy dep, so the bank-start matmul carries only the PSUM
            # WAW wait (one sem-wait slot per matmul)
            nc.tensor.ldweights(o0[g8][:, 0:128])
            for g in range(NPAIR):
                nc.tensor.matmul(
                    ps1[g8][:],
                    lhsT=ct8_ap(g, ot),
                    rhs=bf8_ap(1, g, b2),
                    start=(g == 0),
                    stop=(g == NPAIR - 1),
                    perf_mode=DR,
                )
            nc.scalar.copy(o1[g8][:], ps1[g8][:])
            nc.gpsimd.tensor_copy(scratch[:, 8 + g8 : 9 + g8], o1[g8][0:1, 0:1])
            nc.gpsimd.dma_start(
                out=out_d[(8 + g8) * 128 : (9 + g8) * 128, :], in_=o1[g8][:]
            )

        # --- Correction pass (fp16): the KSEL=128 selected rows are the
        # stationary operand and C the moving one; the output lands as
        # [sel row, out] directly. Two sequential column-half chains so the
        # first half's drain overlaps the second half's matmuls.
        nc.tensor.ldweights(ct16_sb[:, 0:128])  # absorb ct16 DMA wait
        nc.tensor.ldweights(bs16_sb[:, 0:128])  # absorb bs16 DMA wait
        psc = [
            psp.tile([128, 512], FP32, tag="ps", name=f"psc_{hh}") for hh in range(2)
        ]
        oc = outp.tile([128, OC], FP16, tag="osb", name="oc")
        for hh in range(2):
            cs = slice(hh * (OC // 2), (hh + 1) * (OC // 2))
            for k in range(KT):
                nc.tensor.matmul(
                    psc[hh][:, 0 : OC // 2],
                    lhsT=bs16_sb[:, k * KSEL : (k + 1) * KSEL],
                    rhs=ct16_sb[:, k * OC + hh * (OC // 2) : k * OC + (hh + 1) * (OC // 2)],
                    start=(k == 0),
                    stop=(k == KT - 1),
                )
            nc.scalar.copy(oc[:, cs], psc[hh][:, 0 : OC // 2])
            nc.gpsimd.tensor_copy(scratch[:, 16 + hh : 17 + hh], oc[0:1, cs][:, 0:1])
            nc.gpsimd.dma_start(out=outc_d[:, cs], in_=oc[:, cs])

    _cache["nc"] = nc
    return nc


def _make_in_maps(x, coefficients):
    L = _legendre_basis_np(np.asarray(x, dtype=np.float32))  # [8192, 3, 16]
    # exact ||Bfull_row||^2 = prod_d sum_k L[b,d,k]^2: ranks rows by fp8
    # quantization-error magnitude
    pred = (L.astype(np.float64) ** 2).sum(axis=2).prod(axis=1)
    sel, perm = [], []
    for bs in range(NB):
        s = np.sort(np.argsort(-pred[bs * BC : (bs + 1) * BC])[:KSEL])
        mask = np.ones(BC, bool)
        mask[s] = False
        sel.append(s)
        # fp8 row order: unselected rows first, selected last (fp8 skips them)
        perm.append(np.concatenate([np.nonzero(mask)[0], s]))

    CT = np.ascontiguousarray(np.asarray(coefficients, dtype=np.float32).T)
    CT[0, :] = 1.0  # folds both the +1 and the -C[:,0] term (Bfull[:,0]==1)
    CT8 = CT.astype(ml_dtypes.float8_e4m3)  # TRN e4m3 (max 240); |C| < 6
    CT16 = CT.astype(np.float16)

    in_maps = []
    for c in range(NB * NO):
        bs, osh = c % NB, c // NB
        Lb = L[bs * BC : (bs + 1) * BC]  # [BC, 3, 16]
        bfull = np.einsum("bi,bj,bk->ijkb", Lb[:, 0], Lb[:, 1], Lb[:, 2])
        bfull = bfull.reshape(NFEAT, BC)
        bf8 = bfull.astype(ml_dtypes.float8_e4m3)[:, perm[bs]]
        # kt-major partition packs: [KT, 128, cols] -> [128, kt, cols]
        ctp = (
            CT8[:, osh * OC : (osh + 1) * OC]
            .reshape(KT, 128, OC)
            .transpose(1, 0, 2)
        )  # [128, 32, 512]
        # fp8 covers only the BF8N unselected rows (permuted to the front)
        bfp = bf8[:, :BF8N].reshape(KT, 128, BF8N).transpose(1, 0, 2)

        # q8: pair-0 combined halves [128, 2(half), 4(row), 512]
        # half h: rows 0-1 = ct s0,s1 cols h*256:(h+1)*256 (padded to 512),
        #         rows 2-3 = bf s0,s1 batch cols of b2-tile h of half 0
        q8 = np.zeros((128, 2, 4, 512), dtype=ml_dtypes.float8_e4m3)
        for h in range(2):
            q8[:, h, 0:2, 0:256] = ctp[:, 0:2, h * 256 : (h + 1) * 256]
            q8[:, h, 2:4, 0 : BW[h]] = bfp[:, 0:2, h * 512 : h * 512 + BW[h]]
        q8 = q8.reshape(128, 8, 512)

        # cb8: pairs 1..15 [128, pair, 6, 512]: rows 0-1 ct s0,s1; rows 2-5
        # bf (b2-major, s inner) of batch half 0, b2=1 rows padded
        cb8 = np.zeros((128, NPAIR - 1, 6, 512), dtype=ml_dtypes.float8_e4m3)
        for p in range(1, NPAIR):
            cb8[:, p - 1, 0:2, :] = ctp[:, 2 * p : 2 * p + 2, :]
            blk = bfp[:, 2 * p : 2 * p + 2, 0:BH]  # [128, 2(s), 960]
            cb8[:, p - 1, 2:4, 0:512] = blk[:, :, 0:512]
            cb8[:, p - 1, 4:6, 0:448] = blk[:, :, 512:960]
        cb8 = cb8.reshape(128, (NPAIR - 1) * 6, 512)

        # bf8: batch half 1, kt-major, zero-padded to [128, 32, 1024]
        bpk1 = np.zeros((128, KT, 1024), dtype=ml_dtypes.float8_e4m3)
        bpk1[:, :, 0:BH] = bfp[:, :, BH:]

        bsel = np.ascontiguousarray(bfull[:, sel[bs]]).astype(np.float16)
        bspk = np.ascontiguousarray(
            bsel.reshape(KT, 128, KSEL).transpose(1, 0, 2).reshape(128, -1)
        )
        slab16 = CT16[:, osh * OC : (osh + 1) * OC]
        cpk16 = np.ascontiguousarray(
            slab16.reshape(KT, 128, OC).transpose(1, 0, 2).reshape(128, -1)
        )
        in_maps.append(
            {
                "q8": np.ascontiguousarray(q8),
                "cb8": np.ascontiguousarray(cb8),
                "bf8": bpk1,
                "ct16": cpk16,
                "bs16": bspk,
            }
        )
    return in_maps, sel, perm


def _assemble(results, sel, perm):
    out = np.empty((BATCH, OUT), dtype=np.float32)
    for c in range(NB * NO):
        bs, osh = c % NB, c // NB
        o16 = results[c]["out16"]  # [16*128, 512]; slot = pass*8 + ot*2 + b2
        core = np.empty((BF8N, OC), dtype=np.float32)
        for h in range(2):
            for ot in range(4):
                for b2 in range(2):
                    w = BW[b2]
                    slot = h * 8 + ot * 2 + b2
                    blk = o16[slot * 128 : (slot + 1) * 128, 0:w]  # [o, b]
                    core[
                        h * BH + b2 * 512 : h * BH + b2 * 512 + w,
                        ot * 128 : (ot + 1) * 128,
                    ] = blk.T.astype(np.float32)
        out[bs * BC + perm[bs][:BF8N], osh * OC : (osh + 1) * OC] = core
        corr = results[c]["outc16"]  # [sel row, out]
        out[bs * BC + sel[bs], osh * OC : (osh + 1) * OC] = corr.astype(np.float32)
    return out


def _run(x, coefficients, trace=False, **kwargs):
    nc = _build_program()
    in_maps, sel, perm = _make_in_maps(x, coefficients)
    res = run_bass_kernel_spmd(
        nc, in_maps, list(range(NB * NO)), trace=trace, **kwargs
    )
    return _assemble(res.results, sel, perm), res


def kernel(x, coefficients):
    out, _ = _run(x, coefficients)
    return out


# revision 91
# speedup vs baseline: 1.0425x; 1.0236x over previous
"""HCR layer (tensor-product Legendre basis -> dense projection) on 8 trn2 cores.

Math: density[b,o] = 1 + sum_f Bfull[b,f] * C[o,f] - C[o,0]
  where Bfull[b, (i,j,k)] = Li(x0)*Lj(x1)*Lk(x2), orthonormal Legendre on [0,1],
  degree 15 -> 16^3 = 4096 features, batch 8192, out 1024.

Since f_0 == 1 exactly, Bfull[:,0] == 1, so with C[:,0] replaced by 1.0 the
plain matmul Bfull @ C'^T equals the final density (the +1 and the -C[o,0]
fold into the feature-0 column).

Precision plan: the bulk matmul runs in fp8 e4m3 with DoubleRow perf mode
(2 fp8 K-values per partition-cycle -> 2x the fp16 rate, 157 TF/s/core,
measured 214ns per K=256 x 512-col matmul). Plain e4m3 on both operands gives
max rel err 3.4e-2 (tolerance 2e-2); the excess error is concentrated in rows
whose Legendre basis has large norm (x near the domain corners). Per batch
shard, the KSEL=128 rows with the largest row energy prod_d sum_k L[b,d,k]^2
(an exact formula for ||Bfull_row||^2) are recomputed in fp16 in a small
second pass and overwritten at host assembly. Measured residual: max rel err
1.619e-2, bit-identical to the ml_dtypes e4m3 simulation.

Sharding: batch 4-way x out 2-way = 8 cores, no communication.
Per core: [2048 batch, 512 out, 4096 feat] in fp8 (16 DoubleRow matmuls of
K=256 per PSUM bank, 2 batch-half passes of 8 banks, 54.8us at peak) +
[128 sel rows, 512 out, 4096 feat] in fp16 (sel rows stationary, C moving:
2 column-half chains of 32 x 256-col matmuls, 6.8us).

Schedule: one in-order sync-HWDGE queue streams all inputs (a single queue
aggregates the 16 HW DMA engines at ~358 GB/s; each dma_start costs ~650ns
of issue time, so transfers are coarse: one combined ct+bf transfer per
K-pair, pair 0 split in two halves). While the first transfer lands, the PE
runs full-width zeroing junk matmuls that double as accumulation-group
pre-opens for the 8 pass-0 banks (start=False on the data-gated matmuls) and
ramp the PE clock. Pass 0 runs pair-outer (consumes each pair as it lands);
pass 1 runs bank-outer so accumulator stops stagger and each bank's
PSUM->SBUF fp16 copy (scalar) + output DMA (gpsimd SW-DGE, with a 1-elem
gpsimd read absorbing the copy wait) hide behind the matmul stream.
"""

from contextlib import ExitStack

import ml_dtypes
import numpy as np

import concourse.bass as bass
import concourse.mybir as mybir
import concourse.tile as tile
from concourse.bass_utils import run_bass_kernel_spmd

M = 15
NDEG = M + 1            # 16
OUT = 1024
BATCH = 8192
NFEAT = NDEG ** 3       # 4096
NB = 4                  # batch shards
NO = 2                  # out shards
BC = BATCH // NB        # 2048 batch per core
OC = OUT // NO          # 512 out per core
KT = NFEAT // 128       # 32 contraction tiles of 128
NPAIR = KT // 2         # 16 DoubleRow pairs of K=256
KSEL = 128              # corrected rows per batch shard (fp16 pass)
BF8N = BC - KSEL        # 1920: rows the fp8 pass computes (selected rows are
                        # recomputed in fp16 anyway, so fp8 skips them)
BH = BF8N // 2          # 960: batch half per fp8 pass
BW = (512, 448)         # b2-tile widths within a 960 half
FP8 = mybir.dt.float8e4
FP16 = mybir.dt.float16
FP32 = mybir.dt.float32
DR = mybir.MatmulPerfMode.DoubleRow

_cache = {}


class _SplitDrainTileContext(tile.TileContext):
    """TRN2 allows few sem waits per instruction; the default kernel-tail
    drain carries one wait per ticked proc and fails walrus codegen. Split
    the waits across a chain of drains on the sync engine."""

    _MAXW = 1

    def _drain_and_barrier(self, tick_clock, wait_clock):
        from concourse.vector_clock import ScopedClock

        nc = self.nc
        drain0 = nc.sync.drain()
        wait_clock.add_sem_waits(
            drain0.ins, ScopedClock({None: tick_clock.global_clock})
        )
        si = drain0.ins.sync_info
        waits = list(si.on_wait) if si and si.on_wait else []
        if len(waits) > self._MAXW:
            drain0.ins.sync_info = mybir.SyncInfo(
                on_wait=waits[: self._MAXW],
                on_update=list(si.on_update) if si.on_update else [],
            )
            for i in range(self._MAXW, len(waits), self._MAXW):
                d = nc.sync.drain()
                d.ins.sync_info = mybir.SyncInfo(
                    on_wait=waits[i : i + self._MAXW], on_update=[]
                )

        nc.all_engine_barrier()
        assert self.sems is not None
        popped = nc._tile_sem_poison_stack.pop()
        assert popped is self._sem_poison
        # Skip clear_and_free_semaphores + the second barrier: the walrus
        # codegen epilogue zeroes the whole 256-sem file (ids 7-255) on
        # every engine anyway, so the bass-level range-clear and its
        # closing barrier only add ~1.5us to the measured tail.


def _legendre_basis_np(x):
    """Match reference fp32 recurrence exactly. x: [B, D] fp32 -> [B, D, 16]."""
    t = 2.0 * x - 1.0
    ps = [np.ones_like(t), t]
    for k in range(1, M):
        ps.append(((2 * k + 1) * t * ps[k] - k * ps[k - 1]) / (k + 1))
    ps = ps[: M + 1]
    scale = np.sqrt(2.0 * np.arange(M + 1, dtype=x.dtype) + 1.0)
    return np.stack(ps, axis=-1) * scale


def _build_program():
    if "nc" in _cache:
        return _cache["nc"]

    nc = bass.Bass(
        "TRN2", target_bir_lowering=False, debug=False, num_devices=NB * NO
    )

    # Partition-major packed inputs (see _make_in_maps for layouts):
    # q8:  pair-0 combined halves [128, 2, 4, 512]: half h = 2 ct rows
    #      (256 data + 256 pad each) + 2 bf rows; h = (ot-half, b2) pairing
    # cb8: pairs 1..15 combined [128, 15, 6, 512]: 2 ct rows + 4 bf rows
    #      (b2-major) per pair -- one DMA delivers a whole pair
    # bf8: batch-half-1 basis [128, KT, BH] (kt-major)
    # ct16/bs16: kt tiles [128, OC] / [128, KSEL] fp16 at cols kt*OC / kt*KSEL
    q8_d = nc.dram_tensor("q8", [128, 8, 512], FP8, kind="ExternalInput").ap()
    cb8_d = nc.dram_tensor(
        "cb8", [128, (NPAIR - 1) * 6, 512], FP8, kind="ExternalInput"
    ).ap()
    bf8_d = nc.dram_tensor("bf8", [128, KT, BH], FP8, kind="ExternalInput").ap()
    # (q8/cb8 bf rows for the b2=1 tile hold 448 data cols + 64 pad)
    ct16_d = nc.dram_tensor("ct16", [128, KT * OC], FP16, kind="ExternalInput").ap()
    bs16_d = nc.dram_tensor("bs16", [128, KT * KSEL], FP16, kind="ExternalInput").ap()
    # 16 bank dumps [128 out, 512 batch] fp16, g = pass*8 + ot*2 + b2
    out_d = nc.dram_tensor("out16", [16 * 128, 512], FP16, kind="ExternalOutput").ap()
    # correction dump [KSEL rows, 512 out] fp16
    outc_d = nc.dram_tensor("outc16", [KSEL, OC], FP16, kind="ExternalOutput").ap()

    with _SplitDrainTileContext(nc) as tc, ExitStack() as ctx:
        jkp = ctx.enter_context(tc.tile_pool(name="jkp", bufs=1))
        ctp8 = ctx.enter_context(tc.tile_pool(name="ctp8", bufs=NPAIR // 2))
        bfp8 = ctx.enter_context(tc.tile_pool(name="bfp8", bufs=NPAIR // 2 + 1))
        ctp16 = ctx.enter_context(tc.tile_pool(name="ctp16", bufs=1))
        bsp16 = ctx.enter_context(tc.tile_pool(name="bsp16", bufs=1))
        psp = ctx.enter_context(tc.tile_pool(name="psp", bufs=8, space="PSUM"))
        outp = ctx.enter_context(tc.tile_pool(name="outp", bufs=21))

        scratch = outp.tile([1, 32], FP16, tag="scratch", name="scratch", bufs=1)

        # --- PE warmup doubles as PSUM pre-open: full-width zeroing junk
        # matmuls (junk is memset 0) open all 8 pass-0 accumulation banks
        # while the first tiles stream in, so no data-gated matmul pays the
        # accumulation-start cost and the PE clock ramps on real-sized work.
        junk = jkp.tile([128, 512], FP16, tag="junk", name="junk")
        nc.vector.memset(junk[:], 0)

        # --- Input DMAs, all on the sync HWDGE queue (one in-order stream).
        # Each dma_start costs ~650ns of issue time on the queue engine, so
        # DMAs are coarse: 2-pair chunks (ct8 interleaved with bf8 half 0)
        # feed the streaming pass 0 with ~2.2us first-data latency, then the
        # rest of the inputs ride in 4 big transfers. The in-order queue
        # naturally prioritizes the latency-critical stream front. ---
        q8_sb = [
            ctp8.tile([128, 4, 512], FP8, tag="q8", name=f"q8_{h}", bufs=2)
            for h in range(2)
        ]
        cb8_sb = [
            bfp8.tile(
                [128, 6, 512], FP8, tag="cb8", name=f"cb8_{p}", bufs=NPAIR - 1
            )
            for p in range(NPAIR - 1)
        ]
        bf81_sb = [
            bfp8.tile([128, 8, BH], FP8, tag="bf81", name=f"bf81_{c}", bufs=4)
            for c in range(4)
        ]
        ct16_sb = ctp16.tile([128, KT * OC], FP16, tag="ct16", name="ct16", bufs=1)
        bs16_sb = bsp16.tile([128, KT * KSEL], FP16, tag="bs16", name="bs16", bufs=1)

        # A single in-order sync queue aggregates all 16 HW DMA engines at
        # ~358 GB/s; splitting across two HWDGE queues halves per-transfer
        # speed (measured), so everything streams here, latency-critical
        # tiles first. One DMA delivers a whole pair (ct+bf combined), and
        # pair 0 rides in two half-transfers so the first matmul is gated
        # on a single 256KB transfer.
        nc.sync.dma_start(out=q8_sb[0][:], in_=q8_d[:, 0:4, :])
        nc.sync.dma_start(out=q8_sb[1][:], in_=q8_d[:, 4:8, :])
        for p in range(NPAIR - 1):
            nc.sync.dma_start(
                out=cb8_sb[p][:], in_=cb8_d[:, p * 6 : (p + 1) * 6, :]
            )
        for c in range(4):
            nc.sync.dma_start(
                out=bf81_sb[c][:], in_=bf8_d[:, c * 8 : (c + 1) * 8, :]
            )
        nc.sync.dma_start(out=ct16_sb[:], in_=ct16_d[:])
        nc.sync.dma_start(out=bs16_sb[:], in_=bs16_d[:])

        def ct8_ap(g, ot):
            if g == 0:
                return q8_sb[ot // 2][:, 0:2, (ot % 2) * 128 : (ot % 2 + 1) * 128]
            return cb8_sb[g - 1][:, 0:2, ot * 128 : (ot + 1) * 128]

        def bf8_ap(h, g, b2):
            w = BW[b2]
            if h == 0:
                if g == 0:
                    return q8_sb[b2][:, 2:4, 0:w]
                return cb8_sb[g - 1][:, 2 + 2 * b2 : 4 + 2 * b2, 0:w]
            c, j = divmod(g, 4)
            return bf81_sb[c][:, j * 2 : j * 2 + 2, b2 * 512 : b2 * 512 + w]

        # --- Pass 0 (fp8 DoubleRow): pair-outer, consume bf8[0] as it lands.
        ps0 = [
            psp.tile([128, 512], FP32, tag="ps", name=f"ps0_{g}") for g in range(8)
        ]
        # banks ordered by first real use (order0 below) so the pre-open
        # chain only ever delays the bank about to be consumed
        preopened = (0, 2, 4, 6, 1, 3, 5, 7)
        for bank in preopened:
            nc.tensor.matmul(
                ps0[bank][:],
                lhsT=junk[:, 0:128],
                rhs=junk[:, 0:512],
                start=True,
                stop=False,
            )
        # pair 0 follows the quarter arrival order (ctA,bfA,ctB,bfB); later
        # pairs run ot-outer over fully-landed chunks.
        order0 = [(0, 0), (1, 0), (2, 0), (3, 0), (0, 1), (1, 1), (2, 1), (3, 1)]
        for g in range(NPAIR):
            order = order0 if g == 0 else [
                (ot, b2) for ot in range(4) for b2 in range(2)
            ]
            for ot, b2 in order:
                bank = ot * 2 + b2
                nc.tensor.matmul(
                    ps0[bank][:, 0 : BW[b2]],
                    lhsT=ct8_ap(g, ot),
                    rhs=bf8_ap(0, g, b2),
                    start=(g == 0 and bank not in preopened),
                    stop=(g == NPAIR - 1),
                    perf_mode=DR,
                )

        # Drain pass 0: ACT copies PSUM fp32 -> SBUF fp16; a 1-elem gpsimd
        # read absorbs the ACT wait onto the gpsimd stream so the SW-DGE
        # output DMA carries only its queue sem (HWDGE DMAs have one wait
        # slot and the ring wait occupies it).
        o0 = [
            outp.tile([128, 512], FP16, tag="osb", name=f"o0_{g}") for g in range(8)
        ]
        # copies/DMAs stay full 512-wide even for 448-col banks: the spare
        # columns carry junk the host ignores, and a full-width read keeps
        # the PSUM WAR dep a single semaphore for the pass-1 bank starts
        for g in range(8):
            nc.scalar.copy(o0[g][:], ps0[g][:])
            nc.gpsimd.tensor_copy(scratch[:, g : g + 1], o0[g][0:1, 0:1])
            nc.gpsimd.dma_start(
                out=out_d[g * 128 : (g + 1) * 128, :], in_=o0[g][:]
            )

        # --- Pass 1 (fp8 DoubleRow): bank-outer so stops stagger and drains
        # overlap the matmul stream. bf8[1] is resident well before needed.
        nc.tensor.ldweights(bf81_sb[0][:, 0:1, 0:128])  # absorb bf8_1 DMA wait
        ps1 = [
            psp.tile([128, 512], FP32, tag="ps", name=f"ps1_{g}") for g in range(8)
        ]
        o1 = [
            outp.tile([128, 512], FP16, tag="osb", name=f"o1_{g}") for g in range(8)
        ]
        for g8 in range(8):
            ot, b2 = divmod(g8, 2)
            # a throwaway weight load reading the pass-0 drain output absorbs
            # the copy dep, so the bank-start matmul carries only the PSUM
            # WAW wait (one sem-wait slot per matmul)
            nc.tensor.ldweights(o0[g8][:, 0:128])
            w = BW[b2]
            for g in range(NPAIR):
                nc.tensor.matmul(
                    ps1[g8][:, 0:w],
                    lhsT=ct8_ap(g, ot),
                    rhs=bf8_ap(1, g, b2),
                    start=(g == 0),
                    stop=(g == NPAIR - 1),
                    perf_mode=DR,
                )
            nc.scalar.copy(o1[g8][:], ps1[g8][:])
            nc.gpsimd.tensor_copy(scratch[:, 8 + g8 : 9 + g8], o1[g8][0:1, 0:1])
            nc.gpsimd.dma_start(
                out=out_d[(8 + g8) * 128 : (9 + g8) * 128, :], in_=o1[g8][:]
            )

        # --- Correction pass (fp16): the KSEL=128 selected rows are the
        # stationary operand and C the moving one; the output lands as
        # [sel row, out] directly. Two sequential column-half chains so the
        # first half's drain overlaps the second half's matmuls.
        nc.tensor.ldweights(ct16_sb[:, 0:128])  # absorb ct16 DMA wait
        nc.tensor.ldweights(bs16_sb[:, 0:128])  # absorb bs16 DMA wait
        psc = [
            psp.tile([128, 512], FP32, tag="ps", name=f"psc_{hh}") for hh in range(2)
        ]
        oc = outp.tile([128, OC], FP16, tag="osb", name="oc")
        for hh in range(2):
            # absorb the reused bank's drain-copy dep (as in pass 1)
            nc.tensor.ldweights(o1[hh][:, 0:128])
            cs = slice(hh * (OC // 2), (hh + 1) * (OC // 2))
            for k in range(KT):
                nc.tensor.matmul(
                    psc[hh][:, 0 : OC // 2],
                    lhsT=bs16_sb[:, k * KSEL : (k + 1) * KSEL],
                    rhs=ct16_sb[:, k * OC + hh * (OC // 2) : k * OC + (hh + 1) * (OC // 2)],
                    start=(k == 0),
                    stop=(k == KT - 1),
                )
            nc.scalar.copy(oc[:, cs], psc[hh][:, 0 : OC // 2])
            nc.gpsimd.tensor_copy(scratch[:, 16 + hh : 17 + hh], oc[0:1, cs][:, 0:1])
            nc.gpsimd.dma_start(out=outc_d[:, cs], in_=oc[:, cs])

    _cache["nc"] = nc
    return nc


def _make_in_maps(x, coefficients):
    L = _legendre_basis_np(np.asarray(x, dtype=np.float32))  # [8192, 3, 16]
    # exact ||Bfull_row||^2 = prod_d sum_k L[b,d,k]^2: ranks rows by fp8
    # quantization-error magnitude
    pred = (L.astype(np.float64) ** 2).sum(axis=2).prod(axis=1)
    sel, perm = [], []
    for bs in range(NB):
        s = np.sort(np.argsort(-pred[bs * BC : (bs + 1) * BC])[:KSEL])
        mask = np.ones(BC, bool)
        mask[s] = False
        sel.append(s)
        # fp8 row order: unselected rows first, selected last (fp8 skips them)
        perm.append(np.concatenate([np.nonzero(mask)[0], s]))

    CT = np.ascontiguousarray(np.asarray(coefficients, dtype=np.float32).T)
    CT[0, :] = 1.0  # folds both the +1 and the -C[:,0] term (Bfull[:,0]==1)
    CT8 = CT.astype(ml_dtypes.float8_e4m3)  # TRN e4m3 (max 240); |C| < 6
    CT16 = CT.astype(np.float16)

    in_maps = []
    for c in range(NB * NO):
        bs, osh = c % NB, c // NB
        Lb = L[bs * BC : (bs + 1) * BC]  # [BC, 3, 16]
        bfull = np.einsum("bi,bj,bk->ijkb", Lb[:, 0], Lb[:, 1], Lb[:, 2])
        bfull = bfull.reshape(NFEAT, BC)
        bf8 = bfull.astype(ml_dtypes.float8_e4m3)[:, perm[bs]]
        # kt-major partition packs: [KT, 128, cols] -> [128, kt, cols]
        ctp = (
            CT8[:, osh * OC : (osh + 1) * OC]
            .reshape(KT, 128, OC)
            .transpose(1, 0, 2)
        )  # [128, 32, 512]
        # fp8 covers only the BF8N unselected rows (permuted to the front)
        bfp = bf8[:, :BF8N].reshape(KT, 128, BF8N).transpose(1, 0, 2)

        # q8: pair-0 combined halves [128, 2(half), 4(row), 512]
        # half h: rows 0-1 = ct s0,s1 cols h*256:(h+1)*256 (padded to 512),
        #         rows 2-3 = bf s0,s1 batch cols of b2-tile h of half 0
        q8 = np.zeros((128, 2, 4, 512), dtype=ml_dtypes.float8_e4m3)
        for h in range(2):
            q8[:, h, 0:2, 0:256] = ctp[:, 0:2, h * 256 : (h + 1) * 256]
            q8[:, h, 2:4, 0 : BW[h]] = bfp[:, 0:2, h * 512 : h * 512 + BW[h]]
        q8 = q8.reshape(128, 8, 512)

        # cb8: pairs 1..15 [128, pair, 6, 512]: rows 0-1 ct s0,s1; rows 2-5
        # bf (b2-major, s inner) of batch half 0, b2=1 rows padded
        cb8 = np.zeros((128, NPAIR - 1, 6, 512), dtype=ml_dtypes.float8_e4m3)
        for p in range(1, NPAIR):
            cb8[:, p - 1, 0:2, :] = ctp[:, 2 * p : 2 * p + 2, :]
            blk = bfp[:, 2 * p : 2 * p + 2, 0:BH]  # [128, 2(s), 960]
            cb8[:, p - 1, 2:4, 0:512] = blk[:, :, 0:512]
            cb8[:, p - 1, 4:6, 0:448] = blk[:, :, 512:960]
        cb8 = cb8.reshape(128, (NPAIR - 1) * 6, 512)

        # bf8: batch half 1, kt-major [128, 32, 960]
        bpk1 = np.ascontiguousarray(bfp[:, :, BH:])

        bsel = np.ascontiguousarray(bfull[:, sel[bs]]).astype(np.float16)
        bspk = np.ascontiguousarray(
            bsel.reshape(KT, 128, KSEL).transpose(1, 0, 2).reshape(128, -1)
        )
        slab16 = CT16[:, osh * OC : (osh + 1) * OC]
        cpk16 = np.ascontiguousarray(
            slab16.reshape(KT, 128, OC).transpose(1, 0, 2).reshape(128, -1)
        )
        in_maps.append(
            {
                "q8": np.ascontiguousarray(q8),
                "cb8": np.ascontiguousarray(cb8),
                "bf8": bpk1,
                "ct16": cpk16,
                "bs16": bspk,
            }
        )
    return in_maps, sel, perm


def _assemble(results, sel, perm):
    out = np.empty((BATCH, OUT), dtype=np.float32)
    for c in range(NB * NO):
        bs, osh = c % NB, c // NB
        o16 = results[c]["out16"]  # [16*128, 512]; slot = pass*8 + ot*2 + b2
        core = np.empty((BF8N, OC), dtype=np.float32)
        for h in range(2):
            for ot in range(4):
                for b2 in range(2):
                    w = BW[b2]
                    slot = h * 8 + ot * 2 + b2
                    blk = o16[slot * 128 : (slot + 1) * 128, 0:w]  # [o, b]
                    core[
                        h * BH + b2 * 512 : h * BH + b2 * 512 + w,
                        ot * 128 : (ot + 1) * 128,
                    ] = blk.T.astype(np.float32)
        out[bs * BC + perm[bs][:BF8N], osh * OC : (osh + 1) * OC] = core
        corr = results[c]["outc16"]  # [sel row, out]
        out[bs * BC + sel[bs], osh * OC : (osh + 1) * OC] = corr.astype(np.float32)
    return out


def _run(x, coefficients, trace=False, **kwargs):
    nc = _build_program()
    in_maps, sel, perm = _make_in_maps(x, coefficients)
    res = run_bass_kernel_spmd(
        nc, in_maps, list(range(NB * NO)), trace=trace, **kwargs
    )
    return _assemble(res.results, sel, perm), res


def kernel(x, coefficients):
    out, _ = _run(x, coefficients)
    return out
